# revision 4
# baseline (speedup 1.0000x reference)
"""Trainium2 Bass kernel for nn_AttentionLayerEnhance (sparse sliding-window attention).

Problem: B=4, L=S=2048, D_MODEL=512, H=8 heads, WINDOW=32, causal + sliding-window
masks + exponential relative-position decay.  Reference returns (out, attn) where
attn is the full [B,H,L,S] softmax tensor (512 MiB, banded: each row t only
attends to s in [t-16, t+16] after window+causal masks -> <=33 nonzeros/row).

Sharding: 8 cores = (batch b, head-group hg): core c -> b=c//2, heads
[4*(c%2), 4*(c%2)+4).  Every core runs the identical program on its own shard.
Device computes everything: input transposes (PE), QKV projections, banded
scores (only a 160-wide diagonal window per 128-row tile), softmax
(multiplicative decay mask + additive -40 mask, exp on ACT with fused
accumulated denominator), the full attn output incl. zero fills, context
(via PE transpose of the attn window), and the output projection.
Host only: shard/slice inputs, build per-tile mask constants from
attn_mask/gamma, sum the two per-batch partial output projections + bo.
"""

import math
import sys

import numpy as np

if "/opt/trn_rl_repo" not in sys.path:
    sys.path.insert(0, "/opt/trn_rl_repo")

import concourse.bacc as bacc
import concourse.bass as bass
import concourse.mybir as mybir
from concourse import tile
from concourse.bass_utils import run_bass_kernel_spmd
from concourse.masks import make_identity

F32 = mybir.dt.float32

D_MODEL = 512
N_HEADS = 8
WINDOW = 32
B, L_FULL, S_FULL = 4, 2048, 2048
HALF = WINDOW // 2  # 16
WW = 160  # per-128-row-tile band window width (128 + 2*16)

# exec-time info stash for test harnesses
LAST_RESULTS = None


def build_nc(L, S, NH):
    """Build the per-core Bass program.  L=query rows, S=key rows, NH=heads."""
    NT = L // 128  # query row tiles
    NS = S // 128  # key row tiles
    DK = 64
    HP = NH // 2  # head pairs
    ND = D_MODEL // 128  # 4 contraction chunks
    KPAD = 16 + S + 128  # padded s-axis for kT / vT (col c <-> s = c-16)

    nc = bacc.Bacc(None, target_bir_lowering=False)

    xq = nc.dram_tensor("xq", [L, D_MODEL], F32, kind="ExternalInput")
    xk = nc.dram_tensor("xk", [S, D_MODEL], F32, kind="ExternalInput")
    xv = nc.dram_tensor("xv", [S, D_MODEL], F32, kind="ExternalInput")
    wq = nc.dram_tensor("wq", [D_MODEL, NH * DK], F32, kind="ExternalInput")
    wk = nc.dram_tensor("wk", [D_MODEL, NH * DK], F32, kind="ExternalInput")
    wv = nc.dram_tensor("wv", [D_MODEL, NH * DK], F32, kind="ExternalInput")
    wo = nc.dram_tensor("wo", [NH * DK, D_MODEL], F32, kind="ExternalInput")
    bq2 = nc.dram_tensor("bq2", [128, HP], F32, kind="ExternalInput")
    bk2 = nc.dram_tensor("bk2", [128, HP], F32, kind="ExternalInput")
    bvv = nc.dram_tensor("bvv", [NH * DK], F32, kind="ExternalInput")
    dmm = nc.dram_tensor("dmm", [NT, 128, WW], F32, kind="ExternalInput")
    ngm = nc.dram_tensor("ngm", [NT, 128, WW], F32, kind="ExternalInput")

    attn = nc.dram_tensor("attn", [NH, L, S], F32, kind="ExternalOutput")
    outp = nc.dram_tensor("outp", [L, D_MODEL], F32, kind="ExternalOutput")

    with tile.TileContext(nc) as tc:
        with (
            tc.tile_pool(name="const", bufs=1) as cpool,
            tc.tile_pool(name="masks", bufs=1) as mpool,
            tc.tile_pool(name="xin", bufs=3) as xpool,
            tc.tile_pool(name="tin", bufs=1) as tpool,
            tc.tile_pool(name="proj", bufs=1) as ppool,
            tc.tile_pool(name="work", bufs=4) as wpool,
            tc.tile_pool(name="attnT", bufs=6) as apool,
            tc.tile_pool(name="outs", bufs=3) as opool,
            tc.tile_pool(name="psA", bufs=4, space="PSUM") as psA,
            tc.tile_pool(name="psB", bufs=2, space="PSUM") as psB,
            tc.tile_pool(name="psC", bufs=2, space="PSUM") as psC,
        ):
            # ---- constants ----
            ident = cpool.tile([128, 128], F32)
            make_identity(nc, ident[:])
            zeros = cpool.tile([128, S], F32)
            nc.vector.memset(zeros[:], 0.0)

            wq_sb = cpool.tile([128, ND, NH * DK], F32)
            nc.scalar.dma_start(out=wq_sb[:], in_=wq[:].rearrange("(c p) n -> p c n", p=128))
            wk_sb = cpool.tile([128, ND, NH * DK], F32)
            nc.scalar.dma_start(out=wk_sb[:], in_=wk[:].rearrange("(c p) n -> p c n", p=128))
            wv_sb = cpool.tile([128, ND, NH * DK], F32)
            nc.scalar.dma_start(out=wv_sb[:], in_=wv[:].rearrange("(c p) n -> p c n", p=128))
            wo_sb = cpool.tile([128, HP, D_MODEL], F32)
            nc.scalar.dma_start(out=wo_sb[:], in_=wo[:].rearrange("(c p) n -> p c n", p=128))

            bq_sb = cpool.tile([128, HP], F32)
            nc.scalar.dma_start(out=bq_sb[:], in_=bq2[:])
            bk_sb = cpool.tile([128, HP], F32)
            nc.scalar.dma_start(out=bk_sb[:], in_=bk2[:])
            bv_ap = bvv[:]
            bv_bc = cpool.tile([128, NH * DK], F32)
            nc.scalar.dma_start(
                out=bv_bc[:],
                in_=bass.AP(tensor=bv_ap.tensor, offset=bv_ap.offset, ap=[[0, 128]] + list(bv_ap.ap)),
            )

            dm_sb = mpool.tile([128, NT, WW], F32)
            nc.scalar.dma_start(out=dm_sb[:], in_=dmm[:].rearrange("i p c -> p i c"))
            ng_sb = mpool.tile([128, NT, WW], F32)
            nc.scalar.dma_start(out=ng_sb[:], in_=ngm[:].rearrange("i p c -> p i c"))

            # ---- projected tensors (SBUF-resident) ----
            # qT: [64*(h%2) partition, h//2 chunk, t]   (head parity packs partitions)
            qT_sb = ppool.tile([128, HP, L], F32)
            # kT padded: col c <-> s = c - 16 ; scores rhs for tile i = cols [128i, 128i+160)
            kT_sb = ppool.tile([128, HP, KPAD], F32)
            # v natural, -16-row-shifted chunks: v_sb[p, j, h*64+d] = v[s=128j-16+p]
            v_sb = ppool.tile([128, NS + 1, NH * DK], F32)
            # context^T: [64*(h%2) partition, h//2 chunk, t]
            ctxT_sb = ppool.tile([128, HP, L], F32)

            # shared transposed-input scratch: tT[p, c, col] = x[row, c*128+p], col = row (+16 for k/v)
            def load_and_transpose(src, n_tiles, col_off):
                for tt in range(n_tiles):
                    x_tile = xpool.tile([128, D_MODEL], F32, tag="xin")
                    nc.scalar.dma_start(out=x_tile[:], in_=src[tt * 128 : (tt + 1) * 128, :])
                    for c in range(ND):
                        ps = psA.tile([128, 128], F32, tag="t128")
                        nc.tensor.transpose(ps[:], x_tile[:, c * 128 : (c + 1) * 128], ident[:])
                        nc.any.tensor_copy(
                            out=tT[:, c, col_off + tt * 128 : col_off + (tt + 1) * 128],
                            in_=ps[:],
                        )

            # ---------- phase A: transposes + projections ----------
            # q
            TCH = min(512, L)
            tT = tpool.tile([128, ND, KPAD], F32, tag="tT")
            load_and_transpose(xq, NT, 0)
            for hp in range(HP):
                for t4 in range(L // TCH):
                    ps = psB.tile([128, 512], F32, tag="wide")
                    for c in range(ND):
                        nc.tensor.matmul(
                            ps[:, :TCH],
                            wq_sb[:, c, hp * 128 : (hp + 1) * 128],
                            tT[:, c, t4 * TCH : (t4 + 1) * TCH],
                            start=(c == 0),
                            stop=(c == ND - 1),
                        )
                    nc.scalar.activation(
                        out=qT_sb[:, hp, t4 * TCH : (t4 + 1) * TCH],
                        in_=ps[:, :TCH],
                        func=mybir.ActivationFunctionType.Identity,
                        bias=bq_sb[:, hp : hp + 1],
                        scale=1.0,
                    )

            # k
            tT = tpool.tile([128, ND, KPAD], F32, tag="tT")
            nc.vector.memset(tT[:, :, 0:16], 0.0)
            nc.vector.memset(tT[:, :, 16 + S : KPAD], 0.0)
            nc.vector.memset(kT_sb[:, :, 0:16], 0.0)
            nc.vector.memset(kT_sb[:, :, 16 + S : KPAD], 0.0)
            load_and_transpose(xk, NS, 16)
            SCH = min(512, S)
            for hp in range(HP):
                for t4 in range(S // SCH):
                    ps = psB.tile([128, 512], F32, tag="wide")
                    for c in range(ND):
                        nc.tensor.matmul(
                            ps[:, :SCH],
                            wk_sb[:, c, hp * 128 : (hp + 1) * 128],
                            tT[:, c, 16 + t4 * SCH : 16 + (t4 + 1) * SCH],
                            start=(c == 0),
                            stop=(c == ND - 1),
                        )
                    nc.scalar.activation(
                        out=kT_sb[:, hp, 16 + t4 * SCH : 16 + (t4 + 1) * SCH],
                        in_=ps[:, :SCH],
                        func=mybir.ActivationFunctionType.Identity,
                        bias=bk_sb[:, hp : hp + 1],
                        scale=1.0,
                    )

            # v (row-shifted chunks; chunk j needs transposed tiles j-1, j)
            tT = tpool.tile([128, ND, KPAD], F32, tag="tT")
            nc.vector.memset(tT[:, :, 0:16], 0.0)
            nc.vector.memset(tT[:, :, 16 + S : KPAD], 0.0)

            def v_chunk(j):
                ps = psC.tile([128, NH * DK], F32, tag="mid")
                for c in range(ND):
                    nc.tensor.matmul(
                        ps[:],
                        tT[:, c, j * 128 : (j + 1) * 128],
                        wv_sb[:, c, :],
                        start=(c == 0),
                        stop=(c == ND - 1),
                    )
                nc.vector.tensor_tensor(
                    out=v_sb[:, j, :], in0=ps[:], in1=bv_bc[:], op=mybir.AluOpType.add
                )

            for tt in range(NS):
                x_tile = xpool.tile([128, D_MODEL], F32, tag="xin")
                nc.scalar.dma_start(out=x_tile[:], in_=xv[tt * 128 : (tt + 1) * 128, :])
                for c in range(ND):
                    ps = psA.tile([128, 128], F32, tag="t128")
                    nc.tensor.transpose(ps[:], x_tile[:, c * 128 : (c + 1) * 128], ident[:])
                    nc.any.tensor_copy(
                        out=tT[:, c, 16 + tt * 128 : 16 + (tt + 1) * 128], in_=ps[:]
                    )
                v_chunk(tt)  # uses tiles tt-1, tt
            v_chunk(NS)

            # ---------- phase B: banded attention ----------
            for i in range(NT):
                c0 = 128 * i - 16
                lo = max(c0, 0)
                hi = min(c0 + WW, S)
                at1 = {}
                at2 = {}
                for h in range(NH):
                    hp, par = h // 2, (h % 2) * 64
                    sc = psC.tile([128, WW], F32, tag="mid")
                    nc.tensor.matmul(
                        sc[:],
                        qT_sb[par : par + 64, hp, i * 128 : (i + 1) * 128],
                        kT_sb[par : par + 64, hp, i * 128 : i * 128 + WW],
                        start=True,
                        stop=True,
                    )
                    tmpA = wpool.tile([128, WW], F32, tag="tmpA")
                    nc.vector.tensor_tensor(
                        out=tmpA[:], in0=sc[:], in1=dm_sb[:, i, :], op=mybir.AluOpType.mult
                    )
                    nc.vector.tensor_tensor(
                        out=tmpA[:], in0=tmpA[:], in1=ng_sb[:, i, :], op=mybir.AluOpType.add
                    )
                    den = wpool.tile([128, 2], F32, tag="den")
                    pex = wpool.tile([128, WW], F32, tag="pex")
                    nc.scalar.activation(
                        out=pex[:],
                        in_=tmpA[:],
                        func=mybir.ActivationFunctionType.Exp,
                        accum_out=den[:, 0:1],
                    )
                    nc.vector.reciprocal(den[:, 1:2], den[:, 0:1])
                    aw = wpool.tile([128, WW], F32, tag="aw")
                    nc.vector.tensor_scalar_mul(out=aw[:], in0=pex[:], scalar1=den[:, 1:2])

                    # attn output: window + zero fills
                    nc.sync.dma_start(
                        out=attn[h, i * 128 : (i + 1) * 128, lo:hi],
                        in_=aw[:, lo - c0 : hi - c0],
                    )
                    if lo > 0:
                        nc.sync.dma_start(
                            out=attn[h, i * 128 : (i + 1) * 128, 0:lo], in_=zeros[:, 0:lo]
                        )
                    if hi < S:
                        nc.sync.dma_start(
                            out=attn[h, i * 128 : (i + 1) * 128, hi:S], in_=zeros[:, 0 : S - hi]
                        )

                    # attn^T for the context matmul
                    p1 = psA.tile([128, 128], F32, tag="t128")
                    nc.tensor.transpose(p1[:], aw[:, 0:128], ident[:])
                    a1 = apool.tile([128, 128], F32, tag="at1")
                    nc.vector.tensor_copy(out=a1[:], in_=p1[:])
                    p2 = psA.tile([32, 128], F32, tag="t128")
                    nc.tensor.transpose(p2[:], aw[:, 128:WW], ident[:])
                    a2 = apool.tile([32, 128], F32, tag="at2")
                    nc.vector.tensor_copy(out=a2[:], in_=p2[:])
                    at1[h] = a1
                    at2[h] = a2

                for hp in range(HP):
                    cx = psA.tile([128, 128], F32, tag="t128")
                    for par in range(2):
                        h = 2 * hp + par
                        nc.tensor.matmul(
                            cx[par * 64 : (par + 1) * 64, :],
                            v_sb[:, i, h * DK : (h + 1) * DK],
                            at1[h][:],
                            start=True,
                            stop=False,
                        )
                        nc.tensor.matmul(
                            cx[par * 64 : (par + 1) * 64, :],
                            v_sb[0:32, i + 1, h * DK : (h + 1) * DK],
                            at2[h][:],
                            start=False,
                            stop=True,
                        )
                    nc.any.tensor_copy(
                        out=ctxT_sb[:, hp, i * 128 : (i + 1) * 128], in_=cx[:]
                    )

                po = psB.tile([128, 512], F32, tag="wide")
                for hp in range(HP):
                    nc.tensor.matmul(
                        po[:],
                        ctxT_sb[:, hp, i * 128 : (i + 1) * 128],
                        wo_sb[:, hp, :],
                        start=(hp == 0),
                        stop=(hp == HP - 1),
                    )
                ot = opool.tile([128, D_MODEL], F32, tag="outp")
                nc.any.tensor_copy(out=ot[:], in_=po[:])
                nc.sync.dma_start(out=outp[i * 128 : (i + 1) * 128, :], in_=ot[:])

    nc.finalize()
    return nc


def build_masks(attn_mask, gamma, L, S, NT):
    """Per-tile decay/allow masks.  attn_mask: [L,S] bool (True=masked)."""
    am = np.asarray(attn_mask).reshape(attn_mask.shape[-2], attn_mask.shape[-1])
    dm = np.zeros((NT, 128, WW), np.float32)
    ng = np.zeros((NT, 128, WW), np.float32)
    scale = 1.0 / math.sqrt(D_MODEL)
    g = float(np.asarray(gamma).reshape(-1)[0])
    for i in range(NT):
        t = 128 * i + np.arange(128)
        s = 128 * i - 16 + np.arange(WW)
        valid_s = (s >= 0) & (s < S)
        # exact replica of the reference window mask (columns are key index s)
        start = np.clip(s - HALF, 0, None)
        end = np.clip(s + HALF + 1, None, S)
        w = (t[:, None] >= start[None, :]) & (t[:, None] < end[None, :])
        sc = np.clip(s, 0, S - 1)
        allow = w & (~am[t[:, None], sc[None, :]]) & valid_s[None, :]
        rel = np.abs(t[:, None] - s[None, :]).astype(np.float32)
        dm[i] = np.where(allow, np.exp(-g * rel) * scale, 0.0).astype(np.float32)
        ng[i] = np.where(allow, 0.0, -40.0).astype(np.float32)
    return dm, ng


_NC_CACHE = {}


def get_nc(L, S, NH):
    key = (L, S, NH)
    if key not in _NC_CACHE:
        _NC_CACHE[key] = build_nc(L, S, NH)
    return _NC_CACHE[key]


def prepare_in_maps(queries, keys, values, attn_mask, Wq, bq, Wk, bk, Wv, bv, Wo, gamma):
    queries = np.asarray(queries, np.float32)
    keys = np.asarray(keys, np.float32)
    values = np.asarray(values, np.float32)
    Wq, bq = np.asarray(Wq, np.float32), np.asarray(bq, np.float32)
    Wk, bk = np.asarray(Wk, np.float32), np.asarray(bk, np.float32)
    Wv, bv = np.asarray(Wv, np.float32), np.asarray(bv, np.float32)
    Wo = np.asarray(Wo, np.float32)

    _, L, _ = queries.shape
    S = keys.shape[1]
    NT = L // 128
    NH = N_HEADS // 2  # heads per core
    HW = NH * 64  # head width per core (256)

    dm, ng = build_masks(attn_mask, gamma, L, S, NT)

    in_maps = []
    for c in range(8):
        b, hg = c // 2, c % 2
        hs = slice(hg * HW, (hg + 1) * HW)
        in_maps.append(
            {
                "xq": np.ascontiguousarray(queries[b]),
                "xk": np.ascontiguousarray(keys[b]),
                "xv": np.ascontiguousarray(values[b]),
                "wq": np.ascontiguousarray(Wq[:, hs]),
                "wk": np.ascontiguousarray(Wk[:, hs]),
                "wv": np.ascontiguousarray(Wv[:, hs]),
                "wo": np.ascontiguousarray(Wo[hs, :]),
                "bq2": np.ascontiguousarray(bq[hs].reshape(2, 128).T),
                "bk2": np.ascontiguousarray(bk[hs].reshape(2, 128).T),
                "bvv": np.ascontiguousarray(bv[hs]),
                "dmm": dm,
                "ngm": ng,
            }
        )
    return in_maps


def assemble(per_core, bo, Bq, L, S):
    """per_core: list of 8 dicts with 'attn' [4,L,S] and 'outp' [L,D]."""
    NH = N_HEADS // 2
    attn = np.empty((Bq, N_HEADS, L, S), np.float32)
    out = np.empty((Bq, L, D_MODEL), np.float32)
    for c in range(8):
        b, hg = c // 2, c % 2
        attn[b, hg * NH : (hg + 1) * NH] = per_core[c]["attn"]
    bo = np.asarray(bo, np.float32)
    for b in range(Bq):
        out[b] = per_core[2 * b]["outp"] + per_core[2 * b + 1]["outp"] + bo
    return out, attn


def kernel(queries, keys, values, attn_mask, Wq, bq, Wk, bk, Wv, bv, Wo, bo, gamma):
    global LAST_RESULTS
    Bq, L, _ = np.asarray(queries).shape
    S = np.asarray(keys).shape[1]
    nc = get_nc(L, S, N_HEADS // 2)
    in_maps = prepare_in_maps(
        queries, keys, values, attn_mask, Wq, bq, Wk, bk, Wv, bv, Wo, gamma
    )
    res = run_bass_kernel_spmd(nc, in_maps, core_ids=list(range(8)))
    LAST_RESULTS = res
    return assemble(res.results, bo, Bq, L, S)


# revision 12
# speedup vs baseline: 231.3025x; 231.3025x over previous
"""Trainium2 Bass kernel for nn_AttentionLayerEnhance (sparse sliding-window attention).

Problem: B=4, L=S=2048, D_MODEL=512, H=8 heads, WINDOW=32, causal + sliding-window
masks + exponential relative-position decay.  Reference returns (out, attn) where
attn is the full [B,H,L,S] softmax tensor (512 MiB, banded: each row t only
attends to s in [t-16, t+16] after window+causal masks -> <=33 nonzeros/row).

Sharding: 8 cores = (batch b, head-group hg): core c -> b=c//2, heads
[4*(c%2), 4*(c%2)+4).  Every core runs the identical program on its own shard.
Device computes everything: input transposes (PE), QKV projections, banded
scores (only a 160-wide diagonal window per 128-row tile), softmax
(multiplicative decay mask + additive -40 mask, exp on ACT with fused
accumulated denominator), the full attn output incl. zero fills, context
(via PE transpose of the attn window), and the output projection.
Host only: shard/slice inputs, build per-tile mask constants from
attn_mask/gamma, sum the two per-batch partial output projections + bo.
"""

import math
import sys

import numpy as np

if "/opt/trn_rl_repo" not in sys.path:
    sys.path.insert(0, "/opt/trn_rl_repo")

import concourse.bacc as bacc
import concourse.bass as bass
import concourse.mybir as mybir
from concourse import tile
from concourse.bass_utils import run_bass_kernel_spmd
from concourse.masks import make_identity

F32 = mybir.dt.float32

D_MODEL = 512
N_HEADS = 8
WINDOW = 32
B, L_FULL, S_FULL = 4, 2048, 2048
HALF = WINDOW // 2  # 16
WW = 160  # per-128-row-tile band window width (128 + 2*16)

# exec-time info stash for test harnesses
LAST_RESULTS = None


def build_nc(L, S, NH, write_attn=True, phase_b=True):
    """Build the per-core Bass program.  L=query rows, S=key rows, NH=heads."""
    NT = L // 128  # query row tiles
    NS = S // 128  # key row tiles
    DK = 64
    HP = NH // 2  # head pairs
    ND = D_MODEL // 128  # 4 contraction chunks
    KPAD = 16 + S + 128  # padded s-axis for kT / vT (col c <-> s = c-16)

    nc = bacc.Bacc(None, target_bir_lowering=False)

    xq = nc.dram_tensor("xq", [L, D_MODEL], F32, kind="ExternalInput")
    xk = nc.dram_tensor("xk", [S, D_MODEL], F32, kind="ExternalInput")
    xv = nc.dram_tensor("xv", [S, D_MODEL], F32, kind="ExternalInput")
    wq = nc.dram_tensor("wq", [D_MODEL, NH * DK], F32, kind="ExternalInput")
    wk = nc.dram_tensor("wk", [D_MODEL, NH * DK], F32, kind="ExternalInput")
    wv = nc.dram_tensor("wv", [D_MODEL, NH * DK], F32, kind="ExternalInput")
    wo = nc.dram_tensor("wo", [NH * DK, D_MODEL], F32, kind="ExternalInput")
    bq2 = nc.dram_tensor("bq2", [128, HP], F32, kind="ExternalInput")
    bk2 = nc.dram_tensor("bk2", [128, HP], F32, kind="ExternalInput")
    bvv = nc.dram_tensor("bvv", [NH * DK], F32, kind="ExternalInput")
    dmm = nc.dram_tensor("dmm", [NT, 128, WW], F32, kind="ExternalInput")
    ngm = nc.dram_tensor("ngm", [NT, 128, WW], F32, kind="ExternalInput")

    attn = nc.dram_tensor("attn", [NH, L, S], F32, kind="ExternalOutput")
    outp = nc.dram_tensor("outp", [L, D_MODEL], F32, kind="ExternalOutput")

    with tile.TileContext(nc) as tc:
        with (
            tc.tile_pool(name="const", bufs=1) as cpool,
            tc.tile_pool(name="masks", bufs=1) as mpool,
            tc.tile_pool(name="xin", bufs=3) as xpool,
            tc.tile_pool(name="tin", bufs=1) as tpool,
            tc.tile_pool(name="proj", bufs=1) as ppool,
            tc.tile_pool(name="work", bufs=4) as wpool,
            tc.tile_pool(name="attnT", bufs=6) as apool,
            tc.tile_pool(name="outs", bufs=3) as opool,
            tc.tile_pool(name="psA", bufs=4, space="PSUM") as psA,
            tc.tile_pool(name="psB", bufs=2, space="PSUM") as psB,
            tc.tile_pool(name="psC", bufs=2, space="PSUM") as psC,
        ):
            # ---- constants ----
            ident = cpool.tile([128, 128], F32)
            make_identity(nc, ident[:])
            zeros = cpool.tile([128, S], F32)
            nc.vector.memset(zeros[:], 0.0)

            wq_sb = cpool.tile([128, ND, NH * DK], F32)
            nc.scalar.dma_start(out=wq_sb[:], in_=wq[:].rearrange("(c p) n -> p c n", p=128))
            wk_sb = cpool.tile([128, ND, NH * DK], F32)
            nc.scalar.dma_start(out=wk_sb[:], in_=wk[:].rearrange("(c p) n -> p c n", p=128))
            wv_sb = cpool.tile([128, ND, NH * DK], F32)
            nc.scalar.dma_start(out=wv_sb[:], in_=wv[:].rearrange("(c p) n -> p c n", p=128))
            wo_sb = cpool.tile([128, HP, D_MODEL], F32)
            nc.scalar.dma_start(out=wo_sb[:], in_=wo[:].rearrange("(c p) n -> p c n", p=128))

            bq_sb = cpool.tile([128, HP], F32)
            nc.scalar.dma_start(out=bq_sb[:], in_=bq2[:])
            bk_sb = cpool.tile([128, HP], F32)
            nc.scalar.dma_start(out=bk_sb[:], in_=bk2[:])
            bv_ap = bvv[:]
            bv_bc = cpool.tile([128, NH * DK], F32)
            nc.scalar.dma_start(
                out=bv_bc[:],
                in_=bass.AP(tensor=bv_ap.tensor, offset=bv_ap.offset, ap=[[0, 128]] + list(bv_ap.ap)),
            )

            dm_sb = mpool.tile([128, NT, WW], F32)
            nc.scalar.dma_start(out=dm_sb[:], in_=dmm[:].rearrange("i p c -> p i c"))
            ng_sb = mpool.tile([128, NT, WW], F32)
            nc.scalar.dma_start(out=ng_sb[:], in_=ngm[:].rearrange("i p c -> p i c"))



            # ---- projected tensors (SBUF-resident) ----
            # qT: [64*(h%2) partition, h//2 chunk, t]   (head parity packs partitions)
            qT_sb = ppool.tile([128, HP, L], F32)
            # kT padded: col c <-> s = c - 16 ; scores rhs for tile i = cols [128i, 128i+160)
            kT_sb = ppool.tile([128, HP, KPAD], F32)
            # v natural, -16-row-shifted chunks: v_sb[p, j, h*64+d] = v[s=128j-16+p]
            v_sb = ppool.tile([128, NS + 1, NH * DK], F32)
            # context^T: [64*(h%2) partition, h//2 chunk, t]
            ctxT_sb = ppool.tile([128, HP, L], F32)

            # shared transposed-input scratch: tT[p, c, col] = x[row, c*128+p], col = row (+16 for k/v)
            def load_and_transpose(src, n_tiles, col_off):
                for tt in range(n_tiles):
                    x_tile = xpool.tile([128, D_MODEL], F32, tag="xin")
                    nc.scalar.dma_start(out=x_tile[:], in_=src[tt * 128 : (tt + 1) * 128, :])
                    for c in range(ND):
                        ps = psA.tile([128, 128], F32, tag="t128")
                        nc.tensor.transpose(ps[:], x_tile[:, c * 128 : (c + 1) * 128], ident[:])
                        nc.any.tensor_copy(
                            out=tT[:, c, col_off + tt * 128 : col_off + (tt + 1) * 128],
                            in_=ps[:],
                        )

            # ---------- phase A: transposes + projections ----------
            # q
            TCH = min(512, L)
            tT = tpool.tile([128, ND, KPAD], F32, tag="tT")
            load_and_transpose(xq, NT, 0)
            for hp in range(HP):
                for t4 in range(L // TCH):
                    ps = psB.tile([128, 512], F32, tag="wide")
                    for c in range(ND):
                        nc.tensor.matmul(
                            ps[:, :TCH],
                            wq_sb[:, c, hp * 128 : (hp + 1) * 128],
                            tT[:, c, t4 * TCH : (t4 + 1) * TCH],
                            start=(c == 0),
                            stop=(c == ND - 1),
                        )
                    nc.scalar.activation(
                        out=qT_sb[:, hp, t4 * TCH : (t4 + 1) * TCH],
                        in_=ps[:, :TCH],
                        func=mybir.ActivationFunctionType.Identity,
                        bias=bq_sb[:, hp : hp + 1],
                        scale=1.0,
                    )

            # k
            tT = tpool.tile([128, ND, KPAD], F32, tag="tT")
            nc.vector.memset(tT[:, :, 0:16], 0.0)
            nc.vector.memset(tT[:, :, 16 + S : KPAD], 0.0)
            nc.vector.memset(kT_sb[:, :, 0:16], 0.0)
            nc.vector.memset(kT_sb[:, :, 16 + S : KPAD], 0.0)
            load_and_transpose(xk, NS, 16)
            SCH = min(512, S)
            for hp in range(HP):
                for t4 in range(S // SCH):
                    ps = psB.tile([128, 512], F32, tag="wide")
                    for c in range(ND):
                        nc.tensor.matmul(
                            ps[:, :SCH],
                            wk_sb[:, c, hp * 128 : (hp + 1) * 128],
                            tT[:, c, 16 + t4 * SCH : 16 + (t4 + 1) * SCH],
                            start=(c == 0),
                            stop=(c == ND - 1),
                        )
                    nc.scalar.activation(
                        out=kT_sb[:, hp, 16 + t4 * SCH : 16 + (t4 + 1) * SCH],
                        in_=ps[:, :SCH],
                        func=mybir.ActivationFunctionType.Identity,
                        bias=bk_sb[:, hp : hp + 1],
                        scale=1.0,
                    )

            # v (row-shifted chunks; chunk j needs transposed tiles j-1, j)
            tT = tpool.tile([128, ND, KPAD], F32, tag="tT")
            nc.vector.memset(tT[:, :, 0:16], 0.0)
            nc.vector.memset(tT[:, :, 16 + S : KPAD], 0.0)

            def v_chunk(j):
                ps = psC.tile([128, NH * DK], F32, tag="mid")
                for c in range(ND):
                    nc.tensor.matmul(
                        ps[:],
                        tT[:, c, j * 128 : (j + 1) * 128],
                        wv_sb[:, c, :],
                        start=(c == 0),
                        stop=(c == ND - 1),
                    )
                nc.vector.tensor_tensor(
                    out=v_sb[:, j, :], in0=ps[:], in1=bv_bc[:], op=mybir.AluOpType.add
                )

            for tt in range(NS):
                x_tile = xpool.tile([128, D_MODEL], F32, tag="xin")
                nc.scalar.dma_start(out=x_tile[:], in_=xv[tt * 128 : (tt + 1) * 128, :])
                for c in range(ND):
                    ps = psA.tile([128, 128], F32, tag="t128")
                    nc.tensor.transpose(ps[:], x_tile[:, c * 128 : (c + 1) * 128], ident[:])
                    nc.any.tensor_copy(
                        out=tT[:, c, 16 + tt * 128 : 16 + (tt + 1) * 128], in_=ps[:]
                    )
                v_chunk(tt)  # uses tiles tt-1, tt
            v_chunk(NS)

            # ---------- phase B: banded attention ----------
            for i in range(NT if phase_b else 0):
                c0 = 128 * i - 16
                lo = max(c0, 0)
                hi = min(c0 + WW, S)
                at1 = {}
                at2 = {}
                for h in range(NH):
                    hp, par = h // 2, (h % 2) * 64
                    sc = psC.tile([128, WW], F32, tag="mid")
                    nc.tensor.matmul(
                        sc[:],
                        qT_sb[par : par + 64, hp, i * 128 : (i + 1) * 128],
                        kT_sb[par : par + 64, hp, i * 128 : i * 128 + WW],
                        start=True,
                        stop=True,
                    )
                    tmpA = wpool.tile([128, WW], F32, tag="tmpA")
                    nc.vector.tensor_tensor(
                        out=tmpA[:], in0=sc[:], in1=dm_sb[:, i, :], op=mybir.AluOpType.mult
                    )
                    nc.vector.tensor_tensor(
                        out=tmpA[:], in0=tmpA[:], in1=ng_sb[:, i, :], op=mybir.AluOpType.add
                    )
                    den = wpool.tile([128, 2], F32, tag="den")
                    pex = wpool.tile([128, WW], F32, tag="pex")
                    nc.scalar.activation(
                        out=pex[:],
                        in_=tmpA[:],
                        func=mybir.ActivationFunctionType.Exp,
                        accum_out=den[:, 0:1],
                    )
                    nc.vector.reciprocal(den[:, 1:2], den[:, 0:1])
                    aw = wpool.tile([128, WW], F32, tag="aw")
                    nc.vector.tensor_scalar_mul(out=aw[:], in0=pex[:], scalar1=den[:, 1:2])

                    # attn output: band window + off-band zero fills
                    if write_attn:
                        nc.sync.dma_start(
                            out=attn[h, i * 128 : (i + 1) * 128, lo:hi],
                            in_=aw[:, lo - c0 : hi - c0],
                        )
                        if lo > 0:
                            nc.sync.dma_start(
                                out=attn[h, i * 128 : (i + 1) * 128, 0:lo],
                                in_=zeros[:, 0:lo],
                            )
                        if hi < S:
                            nc.sync.dma_start(
                                out=attn[h, i * 128 : (i + 1) * 128, hi:S],
                                in_=zeros[:, 0 : S - hi],
                            )

                    # attn^T for the context matmul
                    p1 = psA.tile([128, 128], F32, tag="t128")
                    nc.tensor.transpose(p1[:], aw[:, 0:128], ident[:])
                    a1 = apool.tile([128, 128], F32, tag="at1")
                    nc.vector.tensor_copy(out=a1[:], in_=p1[:])
                    p2 = psA.tile([32, 128], F32, tag="t128")
                    nc.tensor.transpose(p2[:], aw[:, 128:WW], ident[:])
                    a2 = apool.tile([32, 128], F32, tag="at2")
                    nc.vector.tensor_copy(out=a2[:], in_=p2[:])
                    at1[h] = a1
                    at2[h] = a2

                for hp in range(HP):
                    cx = psA.tile([128, 128], F32, tag="t128")
                    for par in range(2):
                        h = 2 * hp + par
                        nc.tensor.matmul(
                            cx[par * 64 : (par + 1) * 64, :],
                            v_sb[:, i, h * DK : (h + 1) * DK],
                            at1[h][:],
                            start=True,
                            stop=False,
                        )
                        nc.tensor.matmul(
                            cx[par * 64 : (par + 1) * 64, :],
                            v_sb[0:32, i + 1, h * DK : (h + 1) * DK],
                            at2[h][:],
                            start=False,
                            stop=True,
                        )
                    nc.any.tensor_copy(
                        out=ctxT_sb[:, hp, i * 128 : (i + 1) * 128], in_=cx[:]
                    )

                po = psB.tile([128, 512], F32, tag="wide")
                for hp in range(HP):
                    nc.tensor.matmul(
                        po[:],
                        ctxT_sb[:, hp, i * 128 : (i + 1) * 128],
                        wo_sb[:, hp, :],
                        start=(hp == 0),
                        stop=(hp == HP - 1),
                    )
                ot = opool.tile([128, D_MODEL], F32, tag="outp")
                nc.any.tensor_copy(out=ot[:], in_=po[:])
                nc.sync.dma_start(out=outp[i * 128 : (i + 1) * 128, :], in_=ot[:])

    nc.finalize()
    return nc


def build_masks(attn_mask, gamma, L, S, NT):
    """Per-tile decay/allow masks.  attn_mask: [L,S] bool (True=masked)."""
    am = np.asarray(attn_mask).reshape(attn_mask.shape[-2], attn_mask.shape[-1])
    dm = np.zeros((NT, 128, WW), np.float32)
    ng = np.zeros((NT, 128, WW), np.float32)
    scale = 1.0 / math.sqrt(D_MODEL)
    g = float(np.asarray(gamma).reshape(-1)[0])
    for i in range(NT):
        t = 128 * i + np.arange(128)
        s = 128 * i - 16 + np.arange(WW)
        valid_s = (s >= 0) & (s < S)
        # exact replica of the reference window mask (columns are key index s)
        start = np.clip(s - HALF, 0, None)
        end = np.clip(s + HALF + 1, None, S)
        w = (t[:, None] >= start[None, :]) & (t[:, None] < end[None, :])
        sc = np.clip(s, 0, S - 1)
        allow = w & (~am[t[:, None], sc[None, :]]) & valid_s[None, :]
        rel = np.abs(t[:, None] - s[None, :]).astype(np.float32)
        dm[i] = np.where(allow, np.exp(-g * rel) * scale, 0.0).astype(np.float32)
        ng[i] = np.where(allow, 0.0, -40.0).astype(np.float32)
    return dm, ng


_NC_CACHE = {}


def get_nc(L, S, NH):
    key = (L, S, NH)
    if key not in _NC_CACHE:
        _NC_CACHE[key] = build_nc(L, S, NH)
    return _NC_CACHE[key]


def prepare_in_maps(queries, keys, values, attn_mask, Wq, bq, Wk, bk, Wv, bv, Wo, gamma):
    queries = np.asarray(queries, np.float32)
    keys = np.asarray(keys, np.float32)
    values = np.asarray(values, np.float32)
    Wq, bq = np.asarray(Wq, np.float32), np.asarray(bq, np.float32)
    Wk, bk = np.asarray(Wk, np.float32), np.asarray(bk, np.float32)
    Wv, bv = np.asarray(Wv, np.float32), np.asarray(bv, np.float32)
    Wo = np.asarray(Wo, np.float32)

    _, L, _ = queries.shape
    S = keys.shape[1]
    NT = L // 128
    NH = N_HEADS // 2  # heads per core
    HW = NH * 64  # head width per core (256)

    dm, ng = build_masks(attn_mask, gamma, L, S, NT)

    in_maps = []
    for c in range(8):
        b, hg = c // 2, c % 2
        hs = slice(hg * HW, (hg + 1) * HW)
        in_maps.append(
            {
                "xq": np.ascontiguousarray(queries[b]),
                "xk": np.ascontiguousarray(keys[b]),
                "xv": np.ascontiguousarray(values[b]),
                "wq": np.ascontiguousarray(Wq[:, hs]),
                "wk": np.ascontiguousarray(Wk[:, hs]),
                "wv": np.ascontiguousarray(Wv[:, hs]),
                "wo": np.ascontiguousarray(Wo[hs, :]),
                "bq2": np.ascontiguousarray(bq[hs].reshape(2, 128).T),
                "bk2": np.ascontiguousarray(bk[hs].reshape(2, 128).T),
                "bvv": np.ascontiguousarray(bv[hs]),
                "dmm": dm,
                "ngm": ng,
            }
        )
    return in_maps


def assemble(per_core, bo, Bq, L, S):
    """per_core: list of 8 dicts with 'attn' [4,L,S] and 'outp' [L,D]."""
    NH = N_HEADS // 2
    attn = np.empty((Bq, N_HEADS, L, S), np.float32)
    out = np.empty((Bq, L, D_MODEL), np.float32)
    for c in range(8):
        b, hg = c // 2, c % 2
        attn[b, hg * NH : (hg + 1) * NH] = per_core[c]["attn"]
    bo = np.asarray(bo, np.float32)
    for b in range(Bq):
        out[b] = per_core[2 * b]["outp"] + per_core[2 * b + 1]["outp"] + bo
    return out, attn


def kernel(queries, keys, values, attn_mask, Wq, bq, Wk, bk, Wv, bv, Wo, bo, gamma):
    global LAST_RESULTS
    Bq, L, _ = np.asarray(queries).shape
    S = np.asarray(keys).shape[1]
    nc = get_nc(L, S, N_HEADS // 2)
    in_maps = prepare_in_maps(
        queries, keys, values, attn_mask, Wq, bq, Wk, bk, Wv, bv, Wo, gamma
    )
    res = run_bass_kernel_spmd(nc, in_maps, core_ids=list(range(8)))
    LAST_RESULTS = res
    return assemble(res.results, bo, Bq, L, S)
